# revision 12
# baseline (speedup 1.0000x reference)
"""DigitCapsules routing kernel for 8 Trainium2 NeuronCores.

Math: in the reference, u_hat is an explicit broadcast of u_core over the
capsule axis i, so b stays constant along i in every routing iteration,
softmax over i is exactly uniform (1/K), and the whole 3-iteration routing
collapses (exactly, in floating point too) to:

    v[b, i, :] = squash((1/576) * sum_{r,k} x2[b, r, k] * W[b, r, k, :])

broadcast over i = 0..575, where x2 = x.reshape(B, 8, 576).transpose(0, 2, 1).

Sharding: batch dim B=32 across 8 cores, 4 batches per core (data parallel).

v3 design notes (HW-trace driven):
 - W is the STATIONARY matmul operand: 18 exact [128, 128] fp16 tiles per
   core (4*576 = 18*128, no padding), x moving at 16 cols/tile.  Measured
   steady state ~26 ns/tile (LDWEIGHTS fp16 fast-weight-load + N=16 MMs
   pipelined through the PE reorder window).
 - Input split across BOTH HWDGE queues (Sync: W tiles 0-8; Scalar: x pack
   then W tiles 9-17).  A single queue drains ~208 B/ns at 2304 B packets
   (packet-rate-bound ~10 ns/pkt); two run concurrently.
 - One [128, 32] PSUM tile accumulates all 4 batches (two 9-tile
   accumulation groups); the k-diagonal extract (mask multiply + grouped
   reduce, fp16 mask against f32 PSUM) is split per group so the first
   half runs under the second W half's DMA/MM shadow.
 - sel16 / sel4 / mask all ride the x DMA as fp16 (mask is 0/1, exact);
   zero gpsimd ops, no separate small DMAs (a [4, 256 B] DMA cost 1.1 us
   to issue in v2).
 - Squash: scalar_tensor_tensor(accum_out) sum-of-squares straight from
   PSUM, ACT Sqrt in parallel with DVE 1+n, reciprocal, one two-scalar-ptr
   tensor_scalar for v.
 - Output: broadcast via one [4, 128] one-hot matmul to PSUM [128, 288],
   then two half copies + two half DMAs (one per queue) so the second
   DMA's issue overlaps the first's flight.
Fixed costs this cannot touch (measured): ~0.75 us bass preamble + entry
barrier, ~7.4 us NRT postamble (per-engine 256-semaphore clear sweep),
~0.8 us HWDGE first-byte latency per queue, ~0.55 us HBM write receipt.
v1: 20890 ns, v2: 19240 ns.
"""

import numpy as np

import concourse.bacc as bacc
import concourse.mybir as mybir
import concourse.tile as tile
from concourse.bass_utils import run_bass_kernel_spmd

N_CORES = 8
B, C, H, W_ = 32, 8, 24, 24
R = H * W_          # 576 routes
KJ = 128            # fused (k=8, j=16) axis, k-major: kj = k*16 + j
D = 16
NB = B // N_CORES   # 4 batches per core
NT = NB * R // 128  # 18 full W tiles per core
XW = 16             # x columns per tile: (pair-half h, k)
XS_X = NT * XW              # 288: x tiles
XS_S16 = XS_X               # +16: sel16
XS_S4 = XS_S16 + D          # +128: sel4 (partitions 0-3)
XS_MK = XS_S4 + KJ          # +32: diag mask
XTOT = XS_MK + 32           # 464 fp16 cols
RNORM = 1.0 / float(R)
RNORM2 = RNORM * RNORM

_cached_nc = None
_last_in_maps = None


def _build():
    nc = bacc.Bacc(trn_type="TRN2")
    f32 = mybir.dt.float32
    f16 = mybir.dt.float16

    w_h = nc.dram_tensor("w", [128, NT * KJ], f16, kind="ExternalInput")
    x_h = nc.dram_tensor("xc", [128, XTOT], f16, kind="ExternalInput")
    out_h = nc.dram_tensor("out", [NB, R, D], f32, kind="ExternalOutput")

    with tile.TileContext(nc) as tc:
        with (
            tc.tile_pool(name="consts", bufs=1) as consts,
            tc.tile_pool(name="wp", bufs=1) as wp,
            tc.tile_pool(name="gps", bufs=1, space="PSUM") as gps,
            tc.tile_pool(name="tps", bufs=1, space="PSUM") as tps,
            tc.tile_pool(name="vps", bufs=1, space="PSUM") as vps,
            tc.tile_pool(name="sm", bufs=14) as sm,
        ):
            wt0 = wp.tile([128, 9 * KJ], f16)
            wt1 = wp.tile([128, 9 * KJ], f16)
            xst = wp.tile([128, XTOT], f16)

            # x + W_A on the Sync HWDGE queue (x first, it gates every MM);
            # W_B on gpsimd SWDGE — its own queue, and the Scalar HWDGE
            # ring is blocked ~1.6 us by ACT table-load fetches at its head
            # (measured v2/v3).  PE consumes W_B's tiles first.
            nc.sync.dma_start(xst[:], x_h[:])
            nc.sync.dma_start(wt0[:], w_h[:, : 9 * KJ])
            nc.gpsimd.dma_start(wt1[:], w_h[:, 9 * KJ :])

            eps_t = consts.tile([NB, 1], f32)
            nc.vector.memset(eps_t[:], 1e-8)

            # G^T[kj, (half, h, k')] += sum_r W[r, kj] * x2[b, r, k']
            g_ps = gps.tile([128, 32], f32)
            t4 = sm.tile([128, NB], f16)
            for half in (1, 0):
                wt = wt0 if half == 0 else wt1
                for tl in range(9):
                    t = 9 * half + tl
                    nc.tensor.matmul(
                        g_ps[:, 16 * half : 16 * half + 16],
                        wt[:, tl * KJ : (tl + 1) * KJ],
                        xst[:, t * XW : (t + 1) * XW],
                        start=(tl == 0),
                        stop=(tl == 8),
                    )
                # k-diagonal for this half while the other half streams
                pm = sm.tile([128, 16], f32, tag="pm")
                nc.vector.tensor_tensor(
                    pm[:], g_ps[:, 16 * half : 16 * half + 16],
                    xst[:, XS_MK + 16 * half : XS_MK + 16 * half + 16],
                    op=mybir.AluOpType.mult,
                )
                with nc.allow_low_precision("fp16 T4 partials, rel ~5e-4"):
                    nc.vector.tensor_reduce(
                        t4[:, 2 * half : 2 * half + 2],
                        pm[:].rearrange("p (b k) -> p b k", k=8),
                        axis=mybir.AxisListType.X,
                        op=mybir.AluOpType.add,
                    )

            # column-sum over k via one-hot sel16: T[b, j] = sum_k T4[k*16+j, b]
            t_ps = tps.tile([NB, D], f32)
            nc.tensor.matmul(
                t_ps[:], t4[:], xst[:, XS_S16 : XS_S16 + D],
                start=True, stop=True,
            )

            # squash: q = sum_j T^2; n = q/576^2; v = T*(n/576)/((1+n)*sqrt(n+1e-8))
            # q via ACT Square(accum_out) straight from PSUM, then Sqrt
            # back-to-back on the same engine.
            sq = sm.tile([NB, D], f32)
            q = sm.tile([NB, 1], f32)
            nc.scalar.activation(
                sq[:], t_ps[:], mybir.ActivationFunctionType.Square,
                accum_out=q[:],
            )
            s_t = sm.tile([NB, 1], f32)
            nc.scalar.activation(
                s_t[:], q[:], mybir.ActivationFunctionType.Sqrt,
                bias=eps_t[:], scale=RNORM2,
            )
            a1 = sm.tile([NB, 1], f32)
            nc.vector.tensor_scalar(
                out=a1[:], in0=q[:], scalar1=RNORM2, scalar2=1.0,
                op0=mybir.AluOpType.mult, op1=mybir.AluOpType.add,
            )
            npr = sm.tile([NB, 1], f32)
            nc.vector.tensor_scalar_mul(npr[:], q[:], RNORM2 * RNORM)
            den = sm.tile([NB, 1], f32)
            nc.vector.tensor_tensor(
                den[:], s_t[:], a1[:], op=mybir.AluOpType.mult
            )
            m_t = sm.tile([NB, 1], f32)
            nc.vector.reciprocal(m_t[:], den[:])
            v_t = sm.tile([NB, D], f16)
            nc.vector.tensor_scalar(
                out=v_t[:], in0=t_ps[:], scalar1=m_t[:], scalar2=npr[:],
                op0=mybir.AluOpType.mult, op1=mybir.AluOpType.mult,
            )

            # broadcast v over partitions (sel4) and the 18-fold free axis;
            # split in free-dim halves so the first copy/DMA overlaps the
            # second matmul's pipe drain
            vb_ps = vps.tile([128, NT * D], f32)
            dst = out_h[:, :, :].flatten().rearrange(
                "(p c) -> p c", c=NT * D)
            HD = NT * D // 2
            vrh = v_t[:].unsqueeze(1).broadcast_to([NB, NT // 2, D])
            sel4 = xst[0:NB, XS_S4 : XS_S4 + KJ]
            nc.tensor.matmul(
                vb_ps[:, 0:HD], sel4, vrh, start=True, stop=True)
            nc.tensor.matmul(
                vb_ps[:, HD:], sel4, vrh, start=True, stop=True)
            vb0 = sm.tile([128, HD], f32)
            nc.vector.tensor_copy(vb0[:], vb_ps[:, 0:HD])
            nc.sync.dma_start(dst[:, 0:HD], vb0[:])
            vb1 = sm.tile([128, HD], f32)
            nc.vector.tensor_copy(vb1[:], vb_ps[:, HD:])
            nc.scalar.dma_start(dst[:, HD:], vb1[:])

    nc.finalize()
    return nc


def _pack(x, w):
    """Host-side packing: fp16 cast + layout only (no math)."""
    x = np.ascontiguousarray(np.asarray(x), dtype=np.float32)
    w = np.ascontiguousarray(np.asarray(w), dtype=np.float32)
    x2 = x.reshape(B, C, R).transpose(0, 2, 1)      # [B, R, 8]
    wf = w.reshape(B, R, KJ)                        # k-major kj = k*16+j

    p_idx = np.arange(128)
    sel16 = (p_idx[:, None] % 16 == np.arange(D)[None, :])
    sel4 = (p_idx[None, :] // 32 == np.arange(NB)[:, None])
    mask = (np.arange(32)[None, :] % 8 == p_idx[:, None] // 16)

    in_maps = []
    for c in range(N_CORES):
        wcore = wf[c * NB : (c + 1) * NB].reshape(NB * R, KJ)
        w_pack = np.ascontiguousarray(
            wcore.reshape(NT, 128, KJ).transpose(1, 0, 2).reshape(128, NT * KJ)
        ).astype(np.float16)

        x2core = x2[c * NB : (c + 1) * NB]          # [4, 576, 8]
        x_pack = np.zeros((128, XTOT), np.float32)
        for t in range(NT):
            pb = 0 if t < 9 else 2
            rows = t * 128 + p_idx
            bb = rows // R
            rl = rows % R
            for h in (0, 1):
                b = pb + h
                sel = bb == b
                x_pack[sel, t * XW + 8 * h : t * XW + 8 * h + 8] = \
                    x2core[b, rl[sel], :]
        x_pack[:, XS_S16 : XS_S16 + D] = sel16
        x_pack[0:NB, XS_S4 : XS_S4 + KJ] = sel4
        x_pack[:, XS_MK : XS_MK + 32] = mask
        in_maps.append({
            "w": w_pack,
            "xc": x_pack.astype(np.float16),
        })
    return in_maps


def kernel(x, route_weights):
    global _cached_nc, _last_in_maps
    if _cached_nc is None:
        _cached_nc = _build()
    nc = _cached_nc

    in_maps = _pack(x, route_weights)
    _last_in_maps = in_maps

    res = run_bass_kernel_spmd(nc, in_maps, core_ids=list(range(N_CORES)))
    return np.concatenate([r["out"] for r in res.results], axis=0)


# revision 13
# speedup vs baseline: 1.0497x; 1.0497x over previous
"""DigitCapsules routing kernel for 8 Trainium2 NeuronCores.

Math: in the reference, u_hat is an explicit broadcast of u_core over the
capsule axis i, so b stays constant along i in every routing iteration,
softmax over i is exactly uniform (1/K), and the whole 3-iteration routing
collapses (exactly, in floating point too) to:

    v[b, i, :] = squash((1/576) * sum_{r,k} x2[b, r, k] * W[b, r, k, :])

broadcast over i = 0..575, where x2 = x.reshape(B, 8, 576).transpose(0, 2, 1).

Sharding: batch dim B=32 across 8 cores, 4 batches per core (data parallel).

v3 design notes (HW-trace driven):
 - W is the STATIONARY matmul operand: 18 exact [128, 128] fp16 tiles per
   core (4*576 = 18*128, no padding), x moving at 16 cols/tile.  Measured
   steady state ~26 ns/tile (LDWEIGHTS fp16 fast-weight-load + N=16 MMs
   pipelined through the PE reorder window).
 - Input split across BOTH HWDGE queues (Sync: W tiles 0-8; Scalar: x pack
   then W tiles 9-17).  A single queue drains ~208 B/ns at 2304 B packets
   (packet-rate-bound ~10 ns/pkt); two run concurrently.
 - One [128, 32] PSUM tile accumulates all 4 batches (two 9-tile
   accumulation groups); the k-diagonal extract (mask multiply + grouped
   reduce, fp16 mask against f32 PSUM) is split per group so the first
   half runs under the second W half's DMA/MM shadow.
 - sel16 / sel4 / mask all ride the x DMA as fp16 (mask is 0/1, exact);
   zero gpsimd ops, no separate small DMAs (a [4, 256 B] DMA cost 1.1 us
   to issue in v2).
 - Squash: scalar_tensor_tensor(accum_out) sum-of-squares straight from
   PSUM, ACT Sqrt in parallel with DVE 1+n, reciprocal, one two-scalar-ptr
   tensor_scalar for v.
 - Output: broadcast via one [4, 128] one-hot matmul to PSUM [128, 288],
   then two half copies + two half DMAs (one per queue) so the second
   DMA's issue overlaps the first's flight.
Fixed costs this cannot touch (measured): ~0.75 us bass preamble + entry
barrier, ~7.4 us NRT postamble (per-engine 256-semaphore clear sweep),
~0.8 us HWDGE first-byte latency per queue, ~0.55 us HBM write receipt.
v1: 20890 ns, v2: 19240 ns.
"""

import numpy as np

import concourse.bacc as bacc
import concourse.mybir as mybir
import concourse.tile as tile
from concourse.bass_utils import run_bass_kernel_spmd

N_CORES = 8
B, C, H, W_ = 32, 8, 24, 24
R = H * W_          # 576 routes
KJ = 128            # fused (k=8, j=16) axis, k-major: kj = k*16 + j
D = 16
NB = B // N_CORES   # 4 batches per core
NT = NB * R // 128  # 18 full W tiles per core
XW = 16             # x columns per tile: (pair-half h, k)
XS_X = NT * XW              # 288: x tiles
XS_S16 = XS_X               # +16: sel16
XS_S4 = XS_S16 + D          # +128: sel4 (partitions 0-3)
XS_MK = XS_S4 + KJ          # +32: diag mask
XTOT = XS_MK + 32           # 464 fp16 cols
RNORM = 1.0 / float(R)
RNORM2 = RNORM * RNORM

_cached_nc = None
_last_in_maps = None


def _build():
    nc = bacc.Bacc(trn_type="TRN2")
    f32 = mybir.dt.float32
    f16 = mybir.dt.float16

    w_h = nc.dram_tensor("w", [128, NT * KJ], f16, kind="ExternalInput")
    x_h = nc.dram_tensor("xc", [128, XTOT], f16, kind="ExternalInput")
    out_h = nc.dram_tensor("out", [NB, R, D], f32, kind="ExternalOutput")

    with tile.TileContext(nc) as tc:
        with (
            tc.tile_pool(name="consts", bufs=1) as consts,
            tc.tile_pool(name="wp", bufs=1) as wp,
            tc.tile_pool(name="gps", bufs=1, space="PSUM") as gps,
            tc.tile_pool(name="tps", bufs=1, space="PSUM") as tps,
            tc.tile_pool(name="vps", bufs=1, space="PSUM") as vps,
            tc.tile_pool(name="sm", bufs=14) as sm,
        ):
            wt0 = wp.tile([128, 9 * KJ], f16)
            wt1 = wp.tile([128, 9 * KJ], f16)
            xst = wp.tile([128, XTOT], f16)

            # x + W_A on the Sync HWDGE queue (x first, it gates every MM);
            # W_B on gpsimd SWDGE — its own queue, and the Scalar HWDGE
            # ring is blocked ~1.6 us by ACT table-load fetches at its head
            # (measured v2/v3).  PE consumes W_B's tiles first.
            nc.sync.dma_start(xst[:], x_h[:], single_packet=True)
            nc.sync.dma_start(wt0[:], w_h[:, : 9 * KJ], single_packet=True)
            nc.gpsimd.dma_start(wt1[:], w_h[:, 9 * KJ :])

            eps_t = consts.tile([NB, 1], f32)
            nc.vector.memset(eps_t[:], 1e-8)
            # prime the Sqrt ACT table (slot 1) during the DMA wait — left
            # to first use it would load mid-squash (cost 1.28 us, v5)
            warm = consts.tile([NB, 1], f32)
            nc.scalar.activation(
                warm[:], eps_t[:], mybir.ActivationFunctionType.Sqrt
            )

            # G^T[kj, (half, h, k')] += sum_r W[r, kj] * x2[b, r, k']
            g_ps = gps.tile([128, 32], f32)
            t4 = sm.tile([128, NB], f16)
            for half in (1, 0):
                wt = wt0 if half == 0 else wt1
                for tl in range(9):
                    t = 9 * half + tl
                    nc.tensor.matmul(
                        g_ps[:, 16 * half : 16 * half + 16],
                        wt[:, tl * KJ : (tl + 1) * KJ],
                        xst[:, t * XW : (t + 1) * XW],
                        start=(tl == 0),
                        stop=(tl == 8),
                    )
                # k-diagonal for this half while the other half streams
                pm = sm.tile([128, 16], f32, tag="pm")
                nc.vector.tensor_tensor(
                    pm[:], g_ps[:, 16 * half : 16 * half + 16],
                    xst[:, XS_MK + 16 * half : XS_MK + 16 * half + 16],
                    op=mybir.AluOpType.mult,
                )
                with nc.allow_low_precision("fp16 T4 partials, rel ~5e-4"):
                    nc.vector.tensor_reduce(
                        t4[:, 2 * half : 2 * half + 2],
                        pm[:].rearrange("p (b k) -> p b k", k=8),
                        axis=mybir.AxisListType.X,
                        op=mybir.AluOpType.add,
                    )

            # column-sum over k via one-hot sel16: T[b, j] = sum_k T4[k*16+j, b]
            t_ps = tps.tile([NB, D], f32)
            nc.tensor.matmul(
                t_ps[:], t4[:], xst[:, XS_S16 : XS_S16 + D],
                start=True, stop=True,
            )

            # squash: q = sum_j T^2; n = q/576^2; v = T*(n/576)/((1+n)*sqrt(n+1e-8))
            # q via ACT Square(accum_out) straight from PSUM, then Sqrt
            # back-to-back on the same engine.
            sq = sm.tile([NB, D], f32)
            q = sm.tile([NB, 1], f32)
            nc.scalar.activation(
                sq[:], t_ps[:], mybir.ActivationFunctionType.Square,
                accum_out=q[:],
            )
            s_t = sm.tile([NB, 1], f32)
            nc.scalar.activation(
                s_t[:], q[:], mybir.ActivationFunctionType.Sqrt,
                bias=eps_t[:], scale=RNORM2,
            )
            a1 = sm.tile([NB, 1], f32)
            nc.vector.tensor_scalar(
                out=a1[:], in0=q[:], scalar1=RNORM2, scalar2=1.0,
                op0=mybir.AluOpType.mult, op1=mybir.AluOpType.add,
            )
            npr = sm.tile([NB, 1], f32)
            nc.vector.tensor_scalar_mul(npr[:], q[:], RNORM2 * RNORM)
            den = sm.tile([NB, 1], f32)
            nc.vector.tensor_tensor(
                den[:], s_t[:], a1[:], op=mybir.AluOpType.mult
            )
            m_t = sm.tile([NB, 1], f32)
            nc.vector.reciprocal(m_t[:], den[:])
            v_t = sm.tile([NB, D], f16)
            nc.vector.tensor_scalar(
                out=v_t[:], in0=t_ps[:], scalar1=m_t[:], scalar2=npr[:],
                op0=mybir.AluOpType.mult, op1=mybir.AluOpType.mult,
            )

            # broadcast v over partitions (sel4) and the 18-fold free axis;
            # split in free-dim halves so the first copy/DMA overlaps the
            # second matmul's pipe drain
            vb_ps = vps.tile([128, NT * D], f32)
            dst = out_h[:, :, :].flatten().rearrange(
                "(p c) -> p c", c=NT * D)
            HD = NT * D // 2
            vrh = v_t[:].unsqueeze(1).broadcast_to([NB, NT // 2, D])
            sel4 = xst[0:NB, XS_S4 : XS_S4 + KJ]
            nc.tensor.matmul(
                vb_ps[:, 0:HD], sel4, vrh, start=True, stop=True)
            nc.tensor.matmul(
                vb_ps[:, HD:], sel4, vrh, start=True, stop=True)
            vb0 = sm.tile([128, HD], f32)
            nc.vector.tensor_copy(vb0[:], vb_ps[:, 0:HD])
            nc.sync.dma_start(dst[:, 0:HD], vb0[:])
            vb1 = sm.tile([128, HD], f32)
            nc.vector.tensor_copy(vb1[:], vb_ps[:, HD:])
            nc.scalar.dma_start(dst[:, HD:], vb1[:])

    nc.finalize()
    return nc


def _pack(x, w):
    """Host-side packing: fp16 cast + layout only (no math)."""
    x = np.ascontiguousarray(np.asarray(x), dtype=np.float32)
    w = np.ascontiguousarray(np.asarray(w), dtype=np.float32)
    x2 = x.reshape(B, C, R).transpose(0, 2, 1)      # [B, R, 8]
    wf = w.reshape(B, R, KJ)                        # k-major kj = k*16+j

    p_idx = np.arange(128)
    sel16 = (p_idx[:, None] % 16 == np.arange(D)[None, :])
    sel4 = (p_idx[None, :] // 32 == np.arange(NB)[:, None])
    mask = (np.arange(32)[None, :] % 8 == p_idx[:, None] // 16)

    in_maps = []
    for c in range(N_CORES):
        wcore = wf[c * NB : (c + 1) * NB].reshape(NB * R, KJ)
        w_pack = np.ascontiguousarray(
            wcore.reshape(NT, 128, KJ).transpose(1, 0, 2).reshape(128, NT * KJ)
        ).astype(np.float16)

        x2core = x2[c * NB : (c + 1) * NB]          # [4, 576, 8]
        x_pack = np.zeros((128, XTOT), np.float32)
        for t in range(NT):
            pb = 0 if t < 9 else 2
            rows = t * 128 + p_idx
            bb = rows // R
            rl = rows % R
            for h in (0, 1):
                b = pb + h
                sel = bb == b
                x_pack[sel, t * XW + 8 * h : t * XW + 8 * h + 8] = \
                    x2core[b, rl[sel], :]
        x_pack[:, XS_S16 : XS_S16 + D] = sel16
        x_pack[0:NB, XS_S4 : XS_S4 + KJ] = sel4
        x_pack[:, XS_MK : XS_MK + 32] = mask
        in_maps.append({
            "w": w_pack,
            "xc": x_pack.astype(np.float16),
        })
    return in_maps


def kernel(x, route_weights):
    global _cached_nc, _last_in_maps
    if _cached_nc is None:
        _cached_nc = _build()
    nc = _cached_nc

    in_maps = _pack(x, route_weights)
    _last_in_maps = in_maps

    res = run_bass_kernel_spmd(nc, in_maps, core_ids=list(range(N_CORES)))
    return np.concatenate([r["out"] for r in res.results], axis=0)


# revision 17
# speedup vs baseline: 1.1003x; 1.0481x over previous
"""DigitCapsules routing kernel for 8 Trainium2 NeuronCores.

Math: in the reference, u_hat is an explicit broadcast of u_core over the
capsule axis i, so b stays constant along i in every routing iteration,
softmax over i is exactly uniform (1/K), and the whole 3-iteration routing
collapses (exactly, in floating point too) to:

    v[b, i, :] = squash((1/576) * sum_{r,k} x2[b, r, k] * W[b, r, k, :])

broadcast over i = 0..575, where x2 = x.reshape(B, 8, 576).transpose(0, 2, 1).

Sharding: batch dim B=32 across 8 cores, 4 batches per core (data parallel).

v3 design notes (HW-trace driven):
 - W is the STATIONARY matmul operand: 18 exact [128, 128] fp16 tiles per
   core (4*576 = 18*128, no padding), x moving at 16 cols/tile.  Measured
   steady state ~26 ns/tile (LDWEIGHTS fp16 fast-weight-load + N=16 MMs
   pipelined through the PE reorder window).
 - Input split across BOTH HWDGE queues (Sync: W tiles 0-8; Scalar: x pack
   then W tiles 9-17).  A single queue drains ~208 B/ns at 2304 B packets
   (packet-rate-bound ~10 ns/pkt); two run concurrently.
 - One [128, 32] PSUM tile accumulates all 4 batches (two 9-tile
   accumulation groups); the k-diagonal extract (mask multiply + grouped
   reduce, fp16 mask against f32 PSUM) is split per group so the first
   half runs under the second W half's DMA/MM shadow.
 - sel16 / sel4 / mask all ride the x DMA as fp16 (mask is 0/1, exact);
   zero gpsimd ops, no separate small DMAs (a [4, 256 B] DMA cost 1.1 us
   to issue in v2).
 - Squash: scalar_tensor_tensor(accum_out) sum-of-squares straight from
   PSUM, ACT Sqrt in parallel with DVE 1+n, reciprocal, one two-scalar-ptr
   tensor_scalar for v.
 - Output: broadcast via one [4, 128] one-hot matmul to PSUM [128, 288],
   then two half copies + two half DMAs (one per queue) so the second
   DMA's issue overlaps the first's flight.
Fixed costs this cannot touch (measured): ~0.75 us bass preamble + entry
barrier, ~7.4 us NRT postamble (per-engine 256-semaphore clear sweep),
~0.8 us HWDGE first-byte latency per queue, ~0.55 us HBM write receipt.
v1: 20890 ns, v2: 19240 ns.
"""

import numpy as np

import concourse.bacc as bacc
import concourse.mybir as mybir
import concourse.tile as tile
from concourse.bass_utils import run_bass_kernel_spmd

N_CORES = 8
B, C, H, W_ = 32, 8, 24, 24
R = H * W_          # 576 routes
KJ = 128            # fused (k=8, j=16) axis, k-major: kj = k*16 + j
D = 16
NB = B // N_CORES   # 4 batches per core
NT = NB * R // 128  # 18 full W tiles per core
XW = 16             # x columns per tile: (pair-half h, k)
XS_X = NT * XW              # 288: x tiles
XS_S16 = XS_X               # +16: sel16
XS_S4 = XS_S16 + D          # +128: sel4 (partitions 0-3)
XS_MK = XS_S4 + KJ          # +32: diag mask
XTOT = XS_MK + 32           # 464 fp16 cols
RNORM = 1.0 / float(R)
RNORM2 = RNORM * RNORM

_cached_nc = None
_last_in_maps = None


def _build():
    nc = bacc.Bacc(trn_type="TRN2")
    f32 = mybir.dt.float32
    f16 = mybir.dt.float16

    w_h = nc.dram_tensor("w", [128, NT * KJ], f16, kind="ExternalInput")
    x_h = nc.dram_tensor("xc", [128, XTOT], f16, kind="ExternalInput")
    out_h = nc.dram_tensor("out", [NB, R, D], f32, kind="ExternalOutput")

    with tile.TileContext(nc) as tc:
        with (
            tc.tile_pool(name="consts", bufs=1) as consts,
            tc.tile_pool(name="wp", bufs=1) as wp,
            tc.tile_pool(name="gps", bufs=2, space="PSUM") as gps,
            tc.tile_pool(name="tps", bufs=1, space="PSUM") as tps,
            tc.tile_pool(name="vps", bufs=1, space="PSUM") as vps,
            tc.tile_pool(name="sm", bufs=14) as sm,
        ):
            wt = wp.tile([128, NT * KJ], f16)
            xst = wp.tile([128, XTOT], f16)

            # ALL of W as ONE Sync-queue DMA: per-engine row cost is
            # ~(99 ns + bytes/29.5), so one DMA of 4608 B rows drains in
            # ~2.0 us where two 2304 B-row DMAs took 2.84 us.  x rides the
            # gpsimd SWDGE queue (the Scalar HWDGE ring always has the ACT
            # table fetch at its head: +1.3 us head-of-line, measured).
            nc.sync.dma_start(wt[:], w_h[:])
            nc.gpsimd.dma_start(xst[:], x_h[:])

            eps_t = consts.tile([NB, 1], f32)
            nc.vector.memset(eps_t[:], 1e-8)
            # prime the Sqrt ACT table during the DMA wait — left to first
            # use it would load mid-squash (cost 1.28 us, v5)
            warm = consts.tile([NB, 1], f32)
            nc.scalar.activation(
                warm[:], eps_t[:], mybir.ActivationFunctionType.Sqrt
            )

            # G^T[kj, (half, h, k')] += sum_r W[r, kj] * x2[b, r, k']
            # separate PSUM tiles per half: one shared tile makes the
            # second half's MMs wait on the first half's diag read (WAR)
            t4 = sm.tile([128, NB], f16)
            for half in (0, 1):
                g_ps = gps.tile([128, 16], f32, tag="g")
                for tl in range(9):
                    t = 9 * half + tl
                    nc.tensor.matmul(
                        g_ps[:],
                        wt[:, t * KJ : (t + 1) * KJ],
                        xst[:, t * XW : (t + 1) * XW],
                        start=(tl == 0),
                        stop=(tl == 8),
                    )
                # k-diagonal for this half while the other half streams
                pm = sm.tile([128, 16], f32, tag="pm")
                nc.vector.tensor_tensor(
                    pm[:], g_ps[:],
                    xst[:, XS_MK + 16 * half : XS_MK + 16 * half + 16],
                    op=mybir.AluOpType.mult,
                )
                with nc.allow_low_precision("fp16 T4 partials, rel ~5e-4"):
                    nc.vector.tensor_reduce(
                        t4[:, 2 * half : 2 * half + 2],
                        pm[:].rearrange("p (b k) -> p b k", k=8),
                        axis=mybir.AxisListType.X,
                        op=mybir.AluOpType.add,
                    )

            # column-sum over k via one-hot sel16: T[b, j] = sum_k T4[k*16+j, b]
            t_ps = tps.tile([NB, D], f32)
            nc.tensor.matmul(
                t_ps[:], t4[:], xst[:, XS_S16 : XS_S16 + D],
                start=True, stop=True,
            )

            # squash: q = sum_j T^2; n = q/576^2; v = T*(n/576)/((1+n)*sqrt(n+1e-8))
            # q via ACT Square(accum_out) straight from PSUM, then Sqrt
            # back-to-back on the same engine.
            sq = sm.tile([NB, D], f32)
            q = sm.tile([NB, 1], f32)
            nc.scalar.activation(
                sq[:], t_ps[:], mybir.ActivationFunctionType.Square,
                accum_out=q[:],
            )
            s_t = sm.tile([NB, 1], f32)
            nc.scalar.activation(
                s_t[:], q[:], mybir.ActivationFunctionType.Sqrt,
                bias=eps_t[:], scale=RNORM2,
            )
            a1 = sm.tile([NB, 1], f32)
            nc.vector.tensor_scalar(
                out=a1[:], in0=q[:], scalar1=RNORM2, scalar2=1.0,
                op0=mybir.AluOpType.mult, op1=mybir.AluOpType.add,
            )
            npr = sm.tile([NB, 1], f32)
            nc.vector.tensor_scalar_mul(npr[:], q[:], RNORM2 * RNORM)
            den = sm.tile([NB, 1], f32)
            nc.vector.tensor_tensor(
                den[:], s_t[:], a1[:], op=mybir.AluOpType.mult
            )
            m_t = sm.tile([NB, 1], f32)
            nc.vector.reciprocal(m_t[:], den[:])
            v_t = sm.tile([NB, D], f16)
            nc.vector.tensor_scalar(
                out=v_t[:], in0=t_ps[:], scalar1=m_t[:], scalar2=npr[:],
                op0=mybir.AluOpType.mult, op1=mybir.AluOpType.mult,
            )

            # broadcast v over partitions (sel4) and the 18-fold free axis;
            # split in free-dim halves so the first copy/DMA overlaps the
            # second matmul's pipe drain
            vb_ps = vps.tile([128, NT * D], f32)
            dst = out_h[:, :, :].flatten().rearrange(
                "(p c) -> p c", c=NT * D)
            HD = NT * D // 2
            vrh = v_t[:].unsqueeze(1).broadcast_to([NB, NT // 2, D])
            sel4 = xst[0:NB, XS_S4 : XS_S4 + KJ]
            nc.tensor.matmul(
                vb_ps[:, 0:HD], sel4, vrh, start=True, stop=True)
            nc.tensor.matmul(
                vb_ps[:, HD:], sel4, vrh, start=True, stop=True)
            vb0 = sm.tile([128, HD], f32)
            nc.vector.tensor_copy(vb0[:], vb_ps[:, 0:HD])
            nc.sync.dma_start(dst[:, 0:HD], vb0[:])
            vb1 = sm.tile([128, HD], f32)
            nc.vector.tensor_copy(vb1[:], vb_ps[:, HD:])
            nc.scalar.dma_start(dst[:, HD:], vb1[:])

    nc.finalize()
    return nc


def _pack(x, w):
    """Host-side packing: fp16 cast + layout only (no math)."""
    x = np.ascontiguousarray(np.asarray(x), dtype=np.float32)
    w = np.ascontiguousarray(np.asarray(w), dtype=np.float32)
    x2 = x.reshape(B, C, R).transpose(0, 2, 1)      # [B, R, 8]
    wf = w.reshape(B, R, KJ)                        # k-major kj = k*16+j

    p_idx = np.arange(128)
    sel16 = (p_idx[:, None] % 16 == np.arange(D)[None, :])
    sel4 = (p_idx[None, :] // 32 == np.arange(NB)[:, None])
    mask = (np.arange(32)[None, :] % 8 == p_idx[:, None] // 16)

    in_maps = []
    for c in range(N_CORES):
        wcore = wf[c * NB : (c + 1) * NB].reshape(NB * R, KJ)
        w_pack = np.ascontiguousarray(
            wcore.reshape(NT, 128, KJ).transpose(1, 0, 2).reshape(128, NT * KJ)
        ).astype(np.float16)

        x2core = x2[c * NB : (c + 1) * NB]          # [4, 576, 8]
        x_pack = np.zeros((128, XTOT), np.float32)
        for t in range(NT):
            pb = 0 if t < 9 else 2
            rows = t * 128 + p_idx
            bb = rows // R
            rl = rows % R
            for h in (0, 1):
                b = pb + h
                sel = bb == b
                x_pack[sel, t * XW + 8 * h : t * XW + 8 * h + 8] = \
                    x2core[b, rl[sel], :]
        x_pack[:, XS_S16 : XS_S16 + D] = sel16
        x_pack[0:NB, XS_S4 : XS_S4 + KJ] = sel4
        x_pack[:, XS_MK : XS_MK + 32] = mask
        in_maps.append({
            "w": w_pack,
            "xc": x_pack.astype(np.float16),
        })
    return in_maps


def kernel(x, route_weights):
    global _cached_nc, _last_in_maps
    if _cached_nc is None:
        _cached_nc = _build()
    nc = _cached_nc

    in_maps = _pack(x, route_weights)
    _last_in_maps = in_maps

    res = run_bass_kernel_spmd(nc, in_maps, core_ids=list(range(N_CORES)))
    return np.concatenate([r["out"] for r in res.results], axis=0)
